# revision 19
# baseline (speedup 1.0000x reference)
"""EntityNetwork recurrence kernel for 8 Trainium2 NeuronCores.

Sharding: data-parallel over batch (B=64 -> 8 per core). Per core the state
is kept transposed as h_T [E=128 partitions, R=160 free] where r = b*NB + k
(b-major).  The sequential T=128 recurrence runs fully on-chip; only the
per-timestep normalized state is DMA'd out.

Host-side precomputation (cheap, O(T*B*E*E) ~ 0.27 GFLOP total):
  - keys_emb = embeddings[keys]; kV = keys_emb @ Vm ; C2 = kV + U_bias
  - fW[t,b,:] = stories[b,t,:] @ W     (folded with C2 into one matmul operand)
  - gw[t,b,k] = <keys_emb[k], stories[b,t]>  (the  <w_j, s_t>  gate term)
  - gbm[t,r]  = g_bias[k] + gw[t,r] + (mask[b,t]-1)*1e9   (mask folded into
    the sigmoid argument; exact for binary masks)

Device per timestep (all fp32):
  pre   = U^T h  (+ fWC2 selected)            2 matmuls -> PSUM [128,160]
  tmp2  = h * facts_broadcast                 DVE
  z     = ones^T tmp2 + gbm                   2 matmuls -> PSUM [1,160]
  g     = sigmoid(z)                          ACT
  gB    = ones (x) g                          K=1 matmul broadcast
  hh    = lrelu(pre, alpha)                   ACT
  u     = h + hh*gB                           2 DVE
  n2    = ones^T (u*u)                        DVE + matmul
  inv   = 1/sqrt(n2+eps)                      ACT + DVE recip
  h'    = u * (ones (x) inv)                  matmul + DVE
"""

import functools
import os

import numpy as np

B, T, E, NB = 64, 128, 128, 20
NCORES = 8
BL = B // NCORES          # 8 stories per core
R = BL * NB               # 160 independent entities per core
S28 = BL + NB             # 28 = fW rows + C2 rows


@functools.lru_cache(maxsize=2)
def _program(alpha: float):
    from contextlib import ExitStack

    import concourse.bacc as bacc
    import concourse.bass as bass  # noqa: F401
    import concourse.tile as tile
    from concourse import mybir

    f32 = mybir.dt.float32
    AF = mybir.ActivationFunctionType

    # packed constant layouts (single DMA per partition-group):
    #   packa [E, 128+1024+160]: U | stor | h0
    #   packb [S28, T*E+160]:    fwc2 | sel28
    #   packc [1, T*R]:          gbm
    PA = E + T * BL + R + 2  # +2 cols: ones, eps
    PB = T * E + R
    nc = bacc.Bacc("TRN2", target_bir_lowering=False, debug=False)
    d_packa = nc.dram_tensor("packa", [E, PA], f32, kind="ExternalInput")
    d_packb = nc.dram_tensor("packb", [S28, PB], f32, kind="ExternalInput")
    PC = T * R + E  # gbm | ones_row
    d_packc = nc.dram_tensor("packc", [1, PC], f32, kind="ExternalInput")
    d_out = nc.dram_tensor("outd", [T, E, R], f32, kind="ExternalOutput")

    with ExitStack() as ctx:
        tc = ctx.enter_context(tile.TileContext(nc))
        consts = ctx.enter_context(tc.tile_pool(name="consts", bufs=1))
        hpool = ctx.enter_context(tc.tile_pool(name="hpool", bufs=3))
        work = ctx.enter_context(tc.tile_pool(name="work", bufs=2))
        rows = ctx.enter_context(tc.tile_pool(name="rows", bufs=2))
        psum = ctx.enter_context(tc.tile_pool(name="psum", bufs=1, space="PSUM"))

        sb_packa = consts.tile([E, PA], f32)
        nc.sync.dma_start(out=sb_packa, in_=d_packa[:, :])
        sb_packb = consts.tile([S28, PB], f32)
        nc.sync.dma_start(out=sb_packb, in_=d_packb[:, :])
        sb_packc = consts.tile([1, PC], f32)
        nc.sync.dma_start(out=sb_packc, in_=d_packc[:, :])
        sb_gbm = sb_packc[:, 0 : T * R]

        sb_u = sb_packa[:, 0:E]
        sb_stor = sb_packa[:, E : E + T * BL]
        sb_h0 = sb_packa[:, E + T * BL : E + T * BL + R]
        ones_col = sb_packa[:, PA - 2 : PA - 1]      # [E, 1] of 1.0
        eps11 = sb_packa[0:1, PA - 1 : PA]           # [1, 1] of 1e-24
        one11 = sb_packa[0:1, PA - 2 : PA - 1]       # [1, 1] of 1.0
        ones_row = sb_packc[0:1, T * R : T * R + E]  # [1, E] of 1.0
        sb_fwc2 = sb_packb[:, 0 : T * E]
        sb_sel = sb_packb[:, T * E : PB]

        h = hpool.tile([E, R], f32, name="h", tag="h")
        nc.vector.tensor_copy(h, sb_h0)

        for t in range(T):
            # pre-activation: U^T h + (facts@W + kV + U_bias) via selector
            pre = psum.tile([E, R], f32, name="pre", tag="pre")
            nc.tensor.matmul(pre, sb_u, h, start=True, stop=False)
            nc.tensor.matmul(
                pre, sb_fwc2[:, t * E : (t + 1) * E], sb_sel, start=False, stop=True
            )

            # gate pre-sum: z[r] = sum_e h[e,r] * facts[e,b(r)]  (+gbm via PE)
            tmp2 = work.tile([E, R], f32, name="tmp2", tag="tmp2")
            hv = h.rearrange("e (b k) -> e b k", b=BL)
            fb = (
                sb_stor[:, t * BL : (t + 1) * BL]
                .unsqueeze(2)
                .broadcast_to([E, BL, NB])
            )
            nc.vector.tensor_mul(tmp2.rearrange("e (b k) -> e b k", b=BL), hv, fb)

            zrow = psum.tile([1, R], f32, name="zrow", tag="zrow")
            nc.tensor.matmul(zrow, ones_col, tmp2, start=True, stop=False)
            nc.tensor.matmul(
                zrow, one11, sb_gbm[0:1, t * R : (t + 1) * R], start=False, stop=True
            )

            g_row = rows.tile([1, R], f32, name="g_row", tag="g_row")
            nc.scalar.activation(g_row, zrow, AF.Sigmoid)

            gB = psum.tile([E, R], f32, name="gB", tag="gB")
            nc.tensor.matmul(gB, ones_row, g_row, start=True, stop=True)

            hh = work.tile([E, R], f32, name="hh", tag="hh")
            nc.scalar.activation(hh, pre, AF.Prelu, alpha=alpha)

            a = work.tile([E, R], f32, name="a", tag="a")
            nc.vector.tensor_mul(a, hh, gB)
            u = work.tile([E, R], f32, name="u", tag="u")
            nc.vector.tensor_add(u, h, a)

            sq = work.tile([E, R], f32, name="sq", tag="sq")
            nc.vector.tensor_mul(sq, u, u)
            n2row = psum.tile([1, R], f32, name="n2row", tag="n2row")
            nc.tensor.matmul(n2row, ones_col, sq, start=True, stop=True)

            sroot = rows.tile([1, R], f32, name="sroot", tag="sroot")
            nc.scalar.activation(sroot, n2row, AF.Sqrt, bias=eps11)
            inv_row = rows.tile([1, R], f32, name="inv_row", tag="inv_row")
            nc.vector.reciprocal(inv_row, sroot)

            invB = psum.tile([E, R], f32, name="invB", tag="invB")
            nc.tensor.matmul(invB, ones_row, inv_row, start=True, stop=True)

            h_new = hpool.tile([E, R], f32, name="h", tag="h")
            nc.vector.tensor_mul(h_new, u, invB)

            nc.sync.dma_start(out=d_out[t], in_=h_new)
            h = h_new

    nc.compile()
    return nc


def _host_prep(stories, mask, ke, g_bias, U, U_bias, Vm, W):
    """Build the per-core device input maps."""
    C2 = ke @ Vm + U_bias[None, :]  # [NB, E]
    # selector matrix [S28, R]
    sel = np.zeros((S28, R), np.float32)
    for b in range(BL):
        for k in range(NB):
            r = b * NB + k
            sel[b, r] = 1.0
            sel[BL + k, r] = 1.0
    h0 = np.tile(ke.T, (1, BL)).astype(np.float32)  # [E, R], col b*NB+k
    u_dev = np.ascontiguousarray(U, np.float32)

    in_maps = []
    for c in range(NCORES):
        sl = slice(c * BL, (c + 1) * BL)
        st_c = stories[sl]  # [BL, T, E]
        m_c = mask[sl]      # [BL, T]
        fW = np.einsum("bte,ef->tbf", st_c, W)  # [T, BL, E]
        fwc2 = np.concatenate(
            [fW, np.broadcast_to(C2[None], (T, NB, E))], axis=1
        )  # [T, S28, E]
        fwc2_dev = np.ascontiguousarray(
            fwc2.transpose(1, 0, 2).reshape(S28, T * E), np.float32
        )
        gw = np.einsum("ke,bte->tbk", ke, st_c)  # [T, BL, NB]
        gbm = (
            g_bias[None, None, :]
            + gw
            + (m_c.T[:, :, None] - 1.0) * 1e9
        )  # [T, BL, NB]
        gbm_dev = np.ascontiguousarray(gbm.reshape(1, T * R), np.float32)
        stor_dev = np.ascontiguousarray(
            st_c.transpose(2, 1, 0).reshape(E, T * BL), np.float32
        )
        onescol = np.ones((E, 1), np.float32)
        epscol = np.full((E, 1), 1e-24, np.float32)
        packa = np.concatenate([u_dev, stor_dev, h0, onescol, epscol], axis=1)
        packb = np.concatenate([fwc2_dev, sel], axis=1)
        packc = np.concatenate([gbm_dev, np.ones((1, E), np.float32)], axis=1)
        in_maps.append(
            {
                "packa": np.ascontiguousarray(packa, np.float32),
                "packb": np.ascontiguousarray(packb, np.float32),
                "packc": np.ascontiguousarray(packc, np.float32),
            }
        )
    return in_maps


def kernel(
    stories,
    stories_mask,
    keys,
    embeddings,
    g_bias,
    U,
    U_bias,
    Vm,
    W,
    prelu_a,
):
    stories = np.asarray(stories, np.float32)
    mask = np.asarray(stories_mask, np.float32)
    keys = np.asarray(keys).astype(np.int64)
    emb = np.asarray(embeddings, np.float32)
    g_bias = np.asarray(g_bias, np.float32)
    U = np.asarray(U, np.float32)
    U_bias = np.asarray(U_bias, np.float32)
    Vm = np.asarray(Vm, np.float32)
    W = np.asarray(W, np.float32)
    alpha = float(np.asarray(prelu_a))

    ke = emb[keys]  # [NB, E]
    in_maps = _host_prep(stories, mask, ke, g_bias, U, U_bias, Vm, W)

    nc = _program(alpha)
    from concourse.bass_utils import run_bass_kernel_spmd

    trace = bool(int(os.environ.get("KBENCH_TRACE", "0")))
    if trace:
        _ensure_ntff_hook()
    res = run_bass_kernel_spmd(
        nc, in_maps, core_ids=list(range(NCORES)), trace=trace
    )
    if trace and res.exec_time_ns is not None:
        kernel.last_exec_time_ns = res.exec_time_ns
        kernel.last_trace = res.instructions_and_trace
    out = np.empty((B, T, NB, E), np.float32)
    for c in range(NCORES):
        o = res.results[c]["outd"]  # [T, E, R]
        out[c * BL : (c + 1) * BL] = o.reshape(T, E, BL, NB).transpose(2, 0, 3, 1)
    return out


kernel.last_exec_time_ns = None
kernel.last_trace = None


def _ensure_ntff_hook():
    """Register the axon NTFF profiling hook if the antenv shim module is
    missing in this image (the libaxon .so itself supports profiling)."""
    import sys
    import types

    try:
        from antenv.axon_hooks import get_axon_ntff_profile_hook  # noqa: F401

        return
    except ImportError:
        pass
    mod = types.ModuleType("antenv.axon_hooks")
    mod._hook = None

    def set_axon_ntff_profile_hook(h):
        mod._hook = h

    def get_axon_ntff_profile_hook():
        return mod._hook

    mod.set_axon_ntff_profile_hook = set_axon_ntff_profile_hook
    mod.get_axon_ntff_profile_hook = get_axon_ntff_profile_hook
    sys.modules["antenv.axon_hooks"] = mod
    try:
        from trn_agent_boot.trn_boot import _ntff_profile_via_ctypes

        hook = _ntff_profile_via_ctypes("/opt/axon/libaxon_pjrt.so")
        if hook is not None:
            mod._hook = hook
    except Exception:
        pass


# revision 22
# speedup vs baseline: 1.3151x; 1.3151x over previous
"""EntityNetwork recurrence kernel for 8 Trainium2 NeuronCores.

Sharding: data-parallel over batch (B=64 -> 8 per core); per core 160
independent entities r=(b,k) evolve a length-128 state over T=128 steps.

Design (v2, "r-layout"): the heavy per-entity scalars (gate g, inverse norm
iota) live on PARTITIONS so they are cheap per-partition DVE/ACT operands.

State per step t (all fp32):
  h_T  [E=128, R=160]  normalized current state, e-layout (PE stationary)
  u_A [128,128], u_B [32,128]  unnormalized state, r-layout (entities x E)
  iota [128, 2]        per-entity 1/||u|| (col 0: tile A, col 1: tile B)
with the invariant  h_cur = u * iota  and  h_T = (u * iota)^T.

Per timestep:
  pre_r = h @ U + (facts@W + keys@Vm + U_bias)    4 matmuls (lhsT = h_T / sel28)
  G     = h @ facts^T                             2 matmuls  [r, b]
  z     = sum_b (G + gbm) * onehot_b(r)           DVE STT + accum (gbm fold)
  g     = 1/(1+exp(-z))                           ACT Exp + DVE add/recip_fast
  hh*g  = Prelu(g * pre)                          ACT Prelu with scale=g col
  u'    = u * iota + hh*g                         DVE STT (normalize fold)
  n2    = sum_e u'^2                              STT + accum (gpsimd/DVE)
  iota' = exp(-0.5 * ln(n2 + eps))                ACT Ln + Exp
  h_T'  = u'^T @ diag(iota')                      2 transpose-matmuls
  out[t] = h_T'                                   DMA

Host-side precompute: keys_emb gather, facts@W fold, gw gate-bias fold,
mask fold (exact for binary masks), all O(T*B*E*E) ~ 0.27 GFLOP numpy.
ACT only ever uses Exp/Ln/Prelu/Copy -> a single table set, no reloads.
"""

import functools
import os

import numpy as np

B, T, E, NB = 64, 128, 128, 20
NCORES = 8
BL = B // NCORES          # 8 stories per core
R = BL * NB               # 160 entities per core
RA = 128                  # tile A entities
RB = R - RA               # 32 tile B entities
S28 = BL + NB             # 28 = fW rows + C2 rows

# packa [128, PA]: U | stor_T | h0_T | h0_rA | sel8A | gbmA | I128 | eps
PA = E + T * BL + R + E + BL + T + E + 1
# packb [32, PB]: fwc2(28 rows) | sel28A(28) | sel28B(28) | h0_rB | sel8B | gbmB
PB = T * E + RA + RB + E + BL + T


@functools.lru_cache(maxsize=2)
def _program(alpha: float):
    from contextlib import ExitStack

    import concourse.bacc as bacc
    import concourse.tile as tile
    from concourse import mybir

    f32 = mybir.dt.float32
    AF = mybir.ActivationFunctionType
    ALU = mybir.AluOpType

    nc = bacc.Bacc("TRN2", target_bir_lowering=False, debug=False)
    d_packa = nc.dram_tensor("packa", [E, PA], f32, kind="ExternalInput")
    d_packb = nc.dram_tensor("packb", [32, PB], f32, kind="ExternalInput")
    d_out = nc.dram_tensor("outd", [T, E, R], f32, kind="ExternalOutput")

    with ExitStack() as ctx:
        tc = ctx.enter_context(tile.TileContext(nc))
        consts = ctx.enter_context(tc.tile_pool(name="consts", bufs=1))
        hpool = ctx.enter_context(tc.tile_pool(name="hpool", bufs=3))
        upool = ctx.enter_context(tc.tile_pool(name="upool", bufs=2))
        work = ctx.enter_context(tc.tile_pool(name="work", bufs=2))
        psum = ctx.enter_context(tc.tile_pool(name="psum", bufs=1, space="PSUM"))

        sb_packa = consts.tile([E, PA], f32)
        nc.sync.dma_start(out=sb_packa, in_=d_packa[:, :])
        sb_packb = consts.tile([32, PB], f32)
        nc.sync.dma_start(out=sb_packb, in_=d_packb[:, :])

        o = 0
        sb_u = sb_packa[:, o : o + E]; o += E
        sb_stor = sb_packa[:, o : o + T * BL]; o += T * BL
        sb_h0T = sb_packa[:, o : o + R]; o += R
        sb_h0rA = sb_packa[:, o : o + E]; o += E
        sb_sel8A = sb_packa[:, o : o + BL]; o += BL
        sb_gbmA = sb_packa[:, o : o + T]; o += T
        sb_I = sb_packa[:, o : o + E]; o += E
        sb_eps = sb_packa[:, o : o + 1]; o += 1
        assert o == PA

        o = 0
        sb_fwc2 = sb_packb[0:S28, o : o + T * E]; o += T * E
        sb_sel28A = sb_packb[0:S28, o : o + RA]; o += RA
        sb_sel28B = sb_packb[0:S28, o : o + RB]; o += RB
        sb_h0rB = sb_packb[:, o : o + E]; o += E
        sb_sel8B = sb_packb[:, o : o + BL]; o += BL
        sb_gbmB = sb_packb[:, o : o + T]; o += T
        assert o == PB

        # initial state
        h_T = hpool.tile([E, R], f32, name="h_T", tag="hT")
        nc.vector.tensor_copy(h_T, sb_h0T)
        u_A = upool.tile([RA, E], f32, name="u_A", tag="uA")
        nc.vector.tensor_copy(u_A, sb_h0rA)
        u_B = upool.tile([RB, E], f32, name="u_B", tag="uB")
        nc.gpsimd.tensor_copy(u_B, sb_h0rB[0:RB, :])
        iota = upool.tile([RA, 2], f32, name="iota", tag="iota")
        nc.vector.memset(iota, 1.0)

        for t in range(T):
            ts_e = slice(t * E, (t + 1) * E)
            ts_b = slice(t * BL, (t + 1) * BL)

            # ---- pre-activation + gate dot products on PE
            preA = psum.tile([RA, E], f32, name="preA", tag="preA")
            preB = psum.tile([RB, E], f32, name="preB", tag="preB")
            GA = psum.tile([RA, BL], f32, name="GA", tag="GA")
            GB = psum.tile([RB, BL], f32, name="GB", tag="GB")
            nc.tensor.matmul(preA, h_T[:, 0:RA], sb_u, start=True, stop=False)
            nc.tensor.matmul(GA, h_T[:, 0:RA], sb_stor[:, ts_b], start=True, stop=True)
            nc.tensor.matmul(preB, h_T[:, RA:R], sb_u, start=True, stop=False)
            nc.tensor.matmul(GB, h_T[:, RA:R], sb_stor[:, ts_b], start=True, stop=True)
            nc.tensor.matmul(preA, sb_sel28A, sb_fwc2[:, ts_e], start=False, stop=True)
            nc.tensor.matmul(preB, sb_sel28B, sb_fwc2[:, ts_e], start=False, stop=True)

            # ---- gate: z = sum_b (G + gbm) * onehot ; g = sigmoid(z)
            zc = work.tile([RA, 2], f32, name="zc", tag="zc")
            junk8A = work.tile([RA, BL], f32, name="junk8A", tag="junk8A")
            junk8B = work.tile([RB, BL], f32, name="junk8B", tag="junk8B")
            nc.vector.scalar_tensor_tensor(
                out=junk8A, in0=GA, scalar=sb_gbmA[:, t : t + 1], in1=sb_sel8A,
                op0=ALU.add, op1=ALU.mult, accum_out=zc[:, 0:1],
            )
            nc.vector.scalar_tensor_tensor(
                out=junk8B, in0=GB, scalar=sb_gbmB[0:RB, t : t + 1],
                in1=sb_sel8B[0:RB, :],
                op0=ALU.add, op1=ALU.mult, accum_out=zc[0:RB, 1:2],
            )
            ez = work.tile([RA, 2], f32, name="ez", tag="ez")
            nc.scalar.activation(ez, zc, AF.Exp, scale=-1.0)
            g2 = work.tile([RA, 2], f32, name="g2", tag="g2")
            nc.vector.tensor_scalar_add(out=ez, in0=ez, scalar1=1.0)
            nc.vector.reciprocal_approx_fast(g2, ez)

            # ---- gated candidate: hh*g = Prelu(g * pre)
            hhgA = work.tile([RA, E], f32, name="hhgA", tag="hhgA")
            nc.scalar.activation(hhgA, preA, AF.Prelu, scale=g2[:, 0:1], alpha=alpha)
            hhgB = work.tile([RB, E], f32, name="hhgB", tag="hhgB")
            nc.scalar.activation(
                hhgB, preB, AF.Prelu, scale=g2[0:RB, 1:2], alpha=alpha
            )

            # ---- state update u' = u*iota + hh*g ; n2 = sum u'^2
            u_An = upool.tile([RA, E], f32, name="u_A", tag="uA")
            nc.vector.scalar_tensor_tensor(
                out=u_An, in0=u_A, scalar=iota[:, 0:1], in1=hhgA,
                op0=ALU.mult, op1=ALU.add,
            )
            u_Bn = upool.tile([RB, E], f32, name="u_B", tag="uB")
            nc.vector.scalar_tensor_tensor(
                out=u_Bn, in0=u_B, scalar=iota[0:RB, 1:2], in1=hhgB,
                op0=ALU.mult, op1=ALU.add,
            )
            n2c = work.tile([RA, 2], f32, name="n2c", tag="n2c")
            junkA = work.tile([RA, E], f32, name="junkA", tag="junkA")
            junkB = work.tile([RB, E], f32, name="junkB", tag="junkB")
            nc.vector.scalar_tensor_tensor(
                out=junkA, in0=u_An, scalar=1.0, in1=u_An,
                op0=ALU.mult, op1=ALU.mult, accum_out=n2c[:, 0:1],
            )
            nc.vector.scalar_tensor_tensor(
                out=junkB, in0=u_Bn, scalar=1.0, in1=u_Bn,
                op0=ALU.mult, op1=ALU.mult, accum_out=n2c[0:RB, 1:2],
            )

            # ---- iota' = rsqrt(n2 + eps)
            ln2 = work.tile([RA, 2], f32, name="ln2", tag="ln2")
            nc.scalar.activation(ln2, n2c, AF.Ln, bias=sb_eps)
            iota_n = upool.tile([RA, 2], f32, name="iota", tag="iota")
            nc.scalar.activation(iota_n, ln2, AF.Exp, scale=-0.5)

            # ---- h_T' = u'^T @ diag(iota') (transpose + normalize fused)
            dmA = work.tile([RA, RA], f32, name="dmA", tag="dmA")
            nc.vector.tensor_scalar_mul(out=dmA, in0=sb_I, scalar1=iota_n[:, 0:1])
            dmB = work.tile([RB, RB], f32, name="dmB", tag="dmB")
            nc.vector.tensor_scalar_mul(
                out=dmB, in0=sb_I[0:RB, 0:RB], scalar1=iota_n[0:RB, 1:2]
            )
            trA = psum.tile([E, RA], f32, name="trA", tag="trA")
            nc.tensor.matmul(trA, u_An, dmA, start=True, stop=True)
            trB = psum.tile([E, RB], f32, name="trB", tag="trB")
            nc.tensor.matmul(trB, u_Bn, dmB, start=True, stop=True)

            h_Tn = hpool.tile([E, R], f32, name="h_T", tag="hT")
            nc.scalar.copy(h_Tn[:, 0:RA], trA)
            nc.vector.tensor_copy(h_Tn[:, RA:R], trB)

            nc.sync.dma_start(out=d_out[t], in_=h_Tn)

            h_T, u_A, u_B, iota = h_Tn, u_An, u_Bn, iota_n

    nc.compile()
    return nc


def _host_prep(stories, mask, ke, g_bias, U, U_bias, Vm, W):
    """Build the per-core device input maps (packa/packb)."""
    C2 = ke @ Vm + U_bias[None, :]  # [NB, E]
    # selector matrices
    sel28 = np.zeros((S28, R), np.float32)
    sel8 = np.zeros((R, BL), np.float32)
    for b in range(BL):
        for k in range(NB):
            r = b * NB + k
            sel28[b, r] = 1.0
            sel28[BL + k, r] = 1.0
            sel8[r, b] = 1.0
    h0T = np.tile(ke.T, (1, BL)).astype(np.float32)        # [E, R]
    h0r = h0T.T.copy()                                     # [R, E]
    ident = np.eye(E, dtype=np.float32)
    u_dev = np.ascontiguousarray(U, np.float32)
    epscol = np.full((E, 1), 1e-24, np.float32)

    in_maps = []
    for c in range(NCORES):
        sl = slice(c * BL, (c + 1) * BL)
        st_c = stories[sl]  # [BL, T, E]
        m_c = mask[sl]      # [BL, T]
        fW = np.einsum("bte,ef->tbf", st_c, W)  # [T, BL, E]
        fwc2 = np.concatenate(
            [fW, np.broadcast_to(C2[None], (T, NB, E))], axis=1
        )  # [T, S28, E]
        fwc2_dev = np.ascontiguousarray(
            fwc2.transpose(1, 0, 2).reshape(S28, T * E), np.float32
        )
        gw = np.einsum("ke,bte->tbk", ke, st_c)  # [T, BL, NB]
        gbm = (
            g_bias[None, None, :] + gw + (m_c.T[:, :, None] - 1.0) * 1e9
        ).reshape(T, R).T  # [R, T]
        gbm = np.ascontiguousarray(gbm, np.float32)
        stor_dev = np.ascontiguousarray(
            st_c.transpose(2, 1, 0).reshape(E, T * BL), np.float32
        )
        packa = np.concatenate(
            [
                u_dev, stor_dev, h0T, h0r[0:RA], sel8[0:RA],
                gbm[0:RA], ident, epscol,
            ],
            axis=1,
        )
        pb = np.zeros((32, PB), np.float32)
        o = 0
        pb[0:S28, o : o + T * E] = fwc2_dev; o += T * E
        pb[0:S28, o : o + RA] = sel28[:, 0:RA]; o += RA
        pb[0:S28, o : o + RB] = sel28[:, RA:R]; o += RB
        pb[0:RB, o : o + E] = h0r[RA:R]; o += E
        pb[0:RB, o : o + BL] = sel8[RA:R]; o += BL
        pb[0:RB, o : o + T] = gbm[RA:R]; o += T
        assert o == PB
        in_maps.append(
            {
                "packa": np.ascontiguousarray(packa, np.float32),
                "packb": pb,
            }
        )
    return in_maps


def kernel(
    stories,
    stories_mask,
    keys,
    embeddings,
    g_bias,
    U,
    U_bias,
    Vm,
    W,
    prelu_a,
):
    stories = np.asarray(stories, np.float32)
    mask = np.asarray(stories_mask, np.float32)
    keys = np.asarray(keys).astype(np.int64)
    emb = np.asarray(embeddings, np.float32)
    g_bias = np.asarray(g_bias, np.float32)
    U = np.asarray(U, np.float32)
    U_bias = np.asarray(U_bias, np.float32)
    Vm = np.asarray(Vm, np.float32)
    W = np.asarray(W, np.float32)
    alpha = float(np.asarray(prelu_a))

    ke = emb[keys]  # [NB, E]
    in_maps = _host_prep(stories, mask, ke, g_bias, U, U_bias, Vm, W)

    nc = _program(alpha)
    from concourse.bass_utils import run_bass_kernel_spmd

    trace = bool(int(os.environ.get("KBENCH_TRACE", "0")))
    if trace:
        _ensure_ntff_hook()
    res = run_bass_kernel_spmd(
        nc, in_maps, core_ids=list(range(NCORES)), trace=trace
    )
    if trace and res.exec_time_ns is not None:
        kernel.last_exec_time_ns = res.exec_time_ns
        kernel.last_trace = res.instructions_and_trace
    out = np.empty((B, T, NB, E), np.float32)
    for c in range(NCORES):
        o = res.results[c]["outd"]  # [T, E, R]
        out[c * BL : (c + 1) * BL] = o.reshape(T, E, BL, NB).transpose(2, 0, 3, 1)
    return out


kernel.last_exec_time_ns = None
kernel.last_trace = None


def _ensure_ntff_hook():
    """Register the axon NTFF profiling hook if the antenv shim module is
    missing in this image (the libaxon .so itself supports profiling)."""
    import sys
    import types

    try:
        from antenv.axon_hooks import get_axon_ntff_profile_hook  # noqa: F401

        return
    except ImportError:
        pass
    mod = types.ModuleType("antenv.axon_hooks")
    mod._hook = None

    def set_axon_ntff_profile_hook(h):
        mod._hook = h

    def get_axon_ntff_profile_hook():
        return mod._hook

    mod.set_axon_ntff_profile_hook = set_axon_ntff_profile_hook
    mod.get_axon_ntff_profile_hook = get_axon_ntff_profile_hook
    sys.modules["antenv.axon_hooks"] = mod
    try:
        from trn_agent_boot.trn_boot import _ntff_profile_via_ctypes

        hook = _ntff_profile_via_ctypes("/opt/axon/libaxon_pjrt.so")
        if hook is not None:
            mod._hook = hook
    except Exception:
        pass


# revision 28
# speedup vs baseline: 1.6709x; 1.2705x over previous
"""EntityNetwork recurrence kernel for 8 Trainium2 NeuronCores.

Sharding: data-parallel over batch (B=64 -> 8 per core); per core 160
independent entities r=(b,k) evolve a length-128 state over T=128 steps.

Design (v2, "r-layout"): the heavy per-entity scalars (gate g, inverse norm
iota) live on PARTITIONS so they are cheap per-partition DVE/ACT operands.

State per step t (all fp32):
  h_T  [E=128, R=160]  normalized current state, e-layout (PE stationary)
  u_A [128,128], u_B [32,128]  unnormalized state, r-layout (entities x E)
  iota [128, 2]        per-entity 1/||u|| (col 0: tile A, col 1: tile B)
with the invariant  h_cur = u * iota  and  h_T = (u * iota)^T.

Per timestep:
  pre_r = h @ U + (facts@W + keys@Vm + U_bias)    4 matmuls (lhsT = h_T / sel28)
  G     = h @ facts^T                             2 matmuls  [r, b]
  z     = sum_b (G + gbm) * onehot_b(r)           DVE STT + accum (gbm fold)
  g     = 1/(1+exp(-z))                           ACT Exp + DVE add/recip_fast
  hh*g  = Prelu(g * pre)                          ACT Prelu with scale=g col
  u'    = u * iota + hh*g                         DVE STT (normalize fold)
  n2    = sum_e u'^2                              STT + accum (gpsimd/DVE)
  iota' = exp(-0.5 * ln(n2 + eps))                ACT Ln + Exp
  h_T'  = u'^T @ diag(iota')                      2 transpose-matmuls
  out[t] = h_T'                                   DMA

Host-side precompute: keys_emb gather, facts@W fold, gw gate-bias fold,
mask fold (exact for binary masks), all O(T*B*E*E) ~ 0.27 GFLOP numpy.
ACT only ever uses Exp/Ln/Prelu/Copy -> a single table set, no reloads.
"""

import functools
import json
import os

import numpy as np


def _patch_act_tables():
    """Keep every ACT function this kernel uses (Sigmoid, Prelu, Square,
    Copy, Identity) only in the `sigmoid_and_others` table set, so bacc's
    table-load placement keeps ONE resident set and the kernel pays zero
    per-timestep ACT_TABLE_LOADs.  Set ids are untouched (entries keep
    their positions), so walrus/runtime still load the genuine set."""
    import functools as _ft

    import concourse.bacc as _bacc
    import concourse.hw_specs as _hw
    from concourse import mybir as _mb

    if getattr(_patch_act_tables, "_done", False):
        return
    AF = _mb.ActivationFunctionType
    mine = {AF.Sigmoid, AF.Prelu, AF.Square, AF.Copy, AF.Identity}
    orig = _hw.get_activation_tables

    @_ft.cache
    def patched(arch):
        out = {}
        for name, funcs in orig(arch).items():
            out[name] = funcs if name == "sigmoid_and_others" else funcs - mine
        return out

    _hw.get_activation_tables = patched
    _bacc.get_activation_tables = patched
    _patch_act_tables._done = True


B, T, E, NB = 64, 128, 128, 20
NCORES = 8
BL = B // NCORES          # 8 stories per core
R = BL * NB               # 160 entities per core
RA = 128                  # tile A entities
RB = R - RA               # 32 tile B entities
S28 = BL + NB             # 28 = fW rows + C2 rows

# packa [128, PA]: U | stor_T | h0_T | h0_rA | sel8A | gbmA | I128 | eps
PA = E + T * BL + R + E + BL + T + E + 1
# packb [32, PB]: fwc2(28 rows) | sel28A(28) | sel28B(28) | h0_rB | sel8B | gbmB
PB = T * E + RA + RB + E + BL + T


@functools.lru_cache(maxsize=2)
def _program(alpha: float):
    from contextlib import ExitStack

    import concourse.bacc as bacc
    import concourse.tile as tile
    from concourse import mybir

    _patch_act_tables()

    f32 = mybir.dt.float32
    f32r = mybir.dt.float32r
    i32 = mybir.dt.int32
    AF = mybir.ActivationFunctionType
    ALU = mybir.AluOpType
    MAGIC = 0x5F3759DF

    nc = bacc.Bacc("TRN2", target_bir_lowering=False, debug=False)
    d_packa = nc.dram_tensor("packa", [E, PA], f32, kind="ExternalInput")
    d_packb = nc.dram_tensor("packb", [32, PB], f32, kind="ExternalInput")
    d_out = nc.dram_tensor("outd", [T, E, R], f32, kind="ExternalOutput")

    with ExitStack() as ctx:
        tc = ctx.enter_context(tile.TileContext(nc))
        consts = ctx.enter_context(tc.tile_pool(name="consts", bufs=1))
        hpool = ctx.enter_context(tc.tile_pool(name="hpool", bufs=3))
        upool = ctx.enter_context(tc.tile_pool(name="upool", bufs=2))
        work = ctx.enter_context(tc.tile_pool(name="work", bufs=2))
        psum = ctx.enter_context(tc.tile_pool(name="psum", bufs=1, space="PSUM"))

        sb_packa = consts.tile([E, PA], f32)
        nc.sync.dma_start(out=sb_packa, in_=d_packa[:, :])
        sb_packb = consts.tile([32, PB], f32)
        nc.sync.dma_start(out=sb_packb, in_=d_packb[:, :])

        o = 0
        sb_u_f = sb_packa[:, o : o + E]; o += E
        sb_stor_f = sb_packa[:, o : o + T * BL]; o += T * BL
        sb_h0T = sb_packa[:, o : o + R]; o += R
        sb_h0rA = sb_packa[:, o : o + E]; o += E
        sb_sel8A = sb_packa[:, o : o + BL]; o += BL
        sb_gbmA = sb_packa[:, o : o + T]; o += T
        sb_I_f = sb_packa[:, o : o + E]; o += E
        sb_eps = sb_packa[:, o : o + 1]; o += 1
        assert o == PA

        o = 0
        sb_fwc2_f = sb_packb[0:S28, o : o + T * E]; o += T * E
        sb_sel28A_f = sb_packb[0:S28, o : o + RA]; o += RA
        sb_sel28B_f = sb_packb[0:S28, o : o + RB]; o += RB
        sb_h0rB = sb_packb[:, o : o + E]; o += E
        sb_sel8B = sb_packb[:, o : o + BL]; o += BL
        sb_gbmB = sb_packb[:, o : o + T]; o += T
        assert o == PB

        # f32r (single-pass matmul) copies of the constant matmul operands
        sb_u = consts.tile([E, E], f32r, name="sb_u")
        nc.vector.tensor_copy(sb_u, sb_u_f)
        sb_stor = consts.tile([E, T * BL], f32r, name="sb_stor")
        nc.vector.tensor_copy(sb_stor, sb_stor_f)
        sb_I = consts.tile([E, E], f32r, name="sb_I")
        nc.vector.tensor_copy(sb_I, sb_I_f)
        sb_fwc2 = consts.tile([S28, T * E], f32r, name="sb_fwc2")
        nc.vector.tensor_copy(sb_fwc2, sb_fwc2_f)
        sb_sel28A = consts.tile([S28, RA], f32r, name="sb_sel28A")
        nc.vector.tensor_copy(sb_sel28A, sb_sel28A_f)
        sb_sel28B = consts.tile([S28, RB], f32r, name="sb_sel28B")
        nc.vector.tensor_copy(sb_sel28B, sb_sel28B_f)

        # initial state
        h_T = hpool.tile([E, R], f32r, name="h_T", tag="hT")
        nc.vector.tensor_copy(h_T, sb_h0T)
        u_A = upool.tile([RA, E], f32r, name="u_A", tag="uA")
        nc.vector.tensor_copy(u_A, sb_h0rA)
        u_B = upool.tile([RB, E], f32r, name="u_B", tag="uB")
        nc.vector.tensor_copy(u_B, sb_h0rB[0:RB, :])
        iota = upool.tile([RA, 2], f32, name="iota", tag="iota")
        nc.vector.memset(iota, 1.0)

        for t in range(T):
            ts_e = slice(t * E, (t + 1) * E)
            ts_b = slice(t * BL, (t + 1) * BL)

            # ---- pre-activation + gate dot products on PE
            preA = psum.tile([RA, E], f32, name="preA", tag="preA")
            preB = psum.tile([RB, E], f32, name="preB", tag="preB")
            GA = psum.tile([RA, BL], f32, name="GA", tag="GA")
            GB = psum.tile([RB, BL], f32, name="GB", tag="GB")
            nc.tensor.matmul(preA, h_T[:, 0:RA], sb_u, start=True, stop=False)
            nc.tensor.matmul(GA, h_T[:, 0:RA], sb_stor[:, ts_b], start=True, stop=True)
            nc.tensor.matmul(preB, h_T[:, RA:R], sb_u, start=True, stop=False)
            nc.tensor.matmul(GB, h_T[:, RA:R], sb_stor[:, ts_b], start=True, stop=True)
            nc.tensor.matmul(preA, sb_sel28A, sb_fwc2[:, ts_e], start=False, stop=True)
            nc.tensor.matmul(preB, sb_sel28B, sb_fwc2[:, ts_e], start=False, stop=True)

            # ---- gate: z = sum_b (G + gbm) * onehot ; g = sigmoid(z)
            zc = work.tile([RA, 2], f32, name="zc", tag="zc")
            junk8A = work.tile([RA, BL], f32, name="junk8A", tag="junk8A")
            junk8B = work.tile([RB, BL], f32, name="junk8B", tag="junk8B")
            nc.vector.scalar_tensor_tensor(
                out=junk8A, in0=GA, scalar=sb_gbmA[:, t : t + 1], in1=sb_sel8A,
                op0=ALU.add, op1=ALU.mult, accum_out=zc[:, 0:1],
            )
            nc.vector.scalar_tensor_tensor(
                out=junk8B, in0=GB, scalar=sb_gbmB[0:RB, t : t + 1],
                in1=sb_sel8B[0:RB, :],
                op0=ALU.add, op1=ALU.mult, accum_out=zc[0:RB, 1:2],
            )
            g2 = work.tile([RA, 2], f32, name="g2", tag="g2")
            nc.scalar.activation(g2, zc, AF.Sigmoid)

            # ---- gated candidate: hh*g = Prelu(g * pre)
            hhgA = work.tile([RA, E], f32, name="hhgA", tag="hhgA")
            nc.scalar.activation(hhgA, preA, AF.Prelu, scale=g2[:, 0:1], alpha=alpha)
            hhgB = work.tile([RB, E], f32, name="hhgB", tag="hhgB")
            nc.scalar.activation(
                hhgB, preB, AF.Prelu, scale=g2[0:RB, 1:2], alpha=alpha
            )

            # ---- state update u' = u*iota + hh*g ; n2 = sum u'^2
            u_An = upool.tile([RA, E], f32r, name="u_A", tag="uA")
            nc.vector.scalar_tensor_tensor(
                out=u_An, in0=u_A, scalar=iota[:, 0:1], in1=hhgA,
                op0=ALU.mult, op1=ALU.add,
            )
            u_Bn = upool.tile([RB, E], f32r, name="u_B", tag="uB")
            nc.vector.scalar_tensor_tensor(
                out=u_Bn, in0=u_B, scalar=iota[0:RB, 1:2], in1=hhgB,
                op0=ALU.mult, op1=ALU.add,
            )
            n2c = work.tile([RA, 2], f32, name="n2c", tag="n2c")
            junkA = work.tile([RA, E], f32, name="junkA", tag="junkA")
            junkB = work.tile([RB, E], f32, name="junkB", tag="junkB")
            nc.vector.scalar_tensor_tensor(
                out=junkA, in0=u_An, scalar=1.0, in1=u_An,
                op0=ALU.mult, op1=ALU.mult, accum_out=n2c[:, 0:1],
            )
            nc.scalar.activation(
                junkB, u_Bn, AF.Square, accum_out=n2c[0:RB, 1:2]
            )

            # ---- iota' = rsqrt(n2): 0x5f3759df seed + 2 Newton steps (DVE)
            i1 = work.tile([RA, 2], i32, name="i1", tag="i1")
            nc.vector.tensor_scalar(
                out=i1, in0=n2c.bitcast(i32), scalar1=1, scalar2=None,
                op0=ALU.arith_shift_right,
            )
            nc.vector.tensor_scalar(
                out=i1, in0=i1, scalar1=0xFFFFFFFF, scalar2=None,
                op0=ALU.bitwise_xor,
            )
            nc.vector.tensor_scalar(
                out=i1, in0=i1, scalar1=MAGIC + 1, scalar2=None, op0=ALU.add,
            )
            y0 = i1.bitcast(f32)
            mneg = work.tile([RA, 2], f32, name="mneg", tag="mneg")
            nc.vector.tensor_scalar(
                out=mneg, in0=n2c, scalar1=-0.5, scalar2=None, op0=ALU.mult
            )
            tt = work.tile([RA, 2], f32, name="tt", tag="tt")
            nc.vector.tensor_mul(tt, y0, y0)
            nc.vector.tensor_mul(tt, tt, mneg)
            nc.vector.tensor_scalar_add(out=tt, in0=tt, scalar1=1.5)
            y1 = work.tile([RA, 2], f32, name="y1", tag="y1")
            nc.vector.tensor_mul(y1, y0, tt)
            tt2 = work.tile([RA, 2], f32, name="tt2", tag="tt2")
            nc.vector.tensor_mul(tt2, y1, y1)
            nc.vector.tensor_mul(tt2, tt2, mneg)
            nc.vector.tensor_scalar_add(out=tt2, in0=tt2, scalar1=1.5)
            iota_n = upool.tile([RA, 2], f32, name="iota", tag="iota")
            nc.vector.tensor_mul(iota_n, y1, tt2)

            # ---- h_T' = u'^T @ diag(iota') (transpose + normalize fused)
            dmA = work.tile([RA, RA], f32r, name="dmA", tag="dmA")
            nc.vector.tensor_scalar_mul(out=dmA, in0=sb_I, scalar1=iota_n[:, 0:1])
            dmB = work.tile([RB, RB], f32r, name="dmB", tag="dmB")
            nc.vector.tensor_scalar_mul(
                out=dmB, in0=sb_I[0:RB, 0:RB], scalar1=iota_n[0:RB, 1:2]
            )
            trA = psum.tile([E, RA], f32, name="trA", tag="trA")
            nc.tensor.matmul(trA, u_An, dmA, start=True, stop=True)
            trB = psum.tile([E, RB], f32, name="trB", tag="trB")
            nc.tensor.matmul(trB, u_Bn, dmB, start=True, stop=True)

            h_Tn = hpool.tile([E, R], f32r, name="h_T", tag="hT")
            nc.scalar.copy(h_Tn[:, 0:RA], trA)
            nc.vector.tensor_copy(h_Tn[:, RA:R], trB)

            nc.sync.dma_start(out=d_out[t], in_=h_Tn.bitcast(f32))

            h_T, u_A, u_B, iota = h_Tn, u_An, u_Bn, iota_n

    nc.compile()
    return nc


def _host_prep(stories, mask, ke, g_bias, U, U_bias, Vm, W):
    """Build the per-core device input maps (packa/packb)."""
    C2 = ke @ Vm + U_bias[None, :]  # [NB, E]
    # selector matrices
    sel28 = np.zeros((S28, R), np.float32)
    sel8 = np.zeros((R, BL), np.float32)
    for b in range(BL):
        for k in range(NB):
            r = b * NB + k
            sel28[b, r] = 1.0
            sel28[BL + k, r] = 1.0
            sel8[r, b] = 1.0
    h0T = np.tile(ke.T, (1, BL)).astype(np.float32)        # [E, R]
    h0r = h0T.T.copy()                                     # [R, E]
    ident = np.eye(E, dtype=np.float32)
    u_dev = np.ascontiguousarray(U, np.float32)
    epscol = np.full((E, 1), 1e-24, np.float32)

    in_maps = []
    for c in range(NCORES):
        sl = slice(c * BL, (c + 1) * BL)
        st_c = stories[sl]  # [BL, T, E]
        m_c = mask[sl]      # [BL, T]
        fW = np.einsum("bte,ef->tbf", st_c, W)  # [T, BL, E]
        fwc2 = np.concatenate(
            [fW, np.broadcast_to(C2[None], (T, NB, E))], axis=1
        )  # [T, S28, E]
        fwc2_dev = np.ascontiguousarray(
            fwc2.transpose(1, 0, 2).reshape(S28, T * E), np.float32
        )
        gw = np.einsum("ke,bte->tbk", ke, st_c)  # [T, BL, NB]
        gbm = (
            g_bias[None, None, :] + gw + (m_c.T[:, :, None] - 1.0) * 1e9
        ).reshape(T, R).T  # [R, T]
        gbm = np.ascontiguousarray(gbm, np.float32)
        stor_dev = np.ascontiguousarray(
            st_c.transpose(2, 1, 0).reshape(E, T * BL), np.float32
        )
        packa = np.concatenate(
            [
                u_dev, stor_dev, h0T, h0r[0:RA], sel8[0:RA],
                gbm[0:RA], ident, epscol,
            ],
            axis=1,
        )
        pb = np.zeros((32, PB), np.float32)
        o = 0
        pb[0:S28, o : o + T * E] = fwc2_dev; o += T * E
        pb[0:S28, o : o + RA] = sel28[:, 0:RA]; o += RA
        pb[0:S28, o : o + RB] = sel28[:, RA:R]; o += RB
        pb[0:RB, o : o + E] = h0r[RA:R]; o += E
        pb[0:RB, o : o + BL] = sel8[RA:R]; o += BL
        pb[0:RB, o : o + T] = gbm[RA:R]; o += T
        assert o == PB
        in_maps.append(
            {
                "packa": np.ascontiguousarray(packa, np.float32),
                "packb": pb,
            }
        )
    return in_maps


def kernel(
    stories,
    stories_mask,
    keys,
    embeddings,
    g_bias,
    U,
    U_bias,
    Vm,
    W,
    prelu_a,
):
    stories = np.asarray(stories, np.float32)
    mask = np.asarray(stories_mask, np.float32)
    keys = np.asarray(keys).astype(np.int64)
    emb = np.asarray(embeddings, np.float32)
    g_bias = np.asarray(g_bias, np.float32)
    U = np.asarray(U, np.float32)
    U_bias = np.asarray(U_bias, np.float32)
    Vm = np.asarray(Vm, np.float32)
    W = np.asarray(W, np.float32)
    alpha = float(np.asarray(prelu_a))

    ke = emb[keys]  # [NB, E]
    in_maps = _host_prep(stories, mask, ke, g_bias, U, U_bias, Vm, W)

    nc = _program(alpha)
    from concourse.bass_utils import run_bass_kernel_spmd

    trace = bool(int(os.environ.get("KBENCH_TRACE", "0")))
    if trace:
        _ensure_ntff_hook()
    res = run_bass_kernel_spmd(
        nc, in_maps, core_ids=list(range(NCORES)), trace=trace
    )
    if trace and res.exec_time_ns is not None:
        kernel.last_exec_time_ns = res.exec_time_ns
        kernel.last_trace = res.instructions_and_trace
    out = np.empty((B, T, NB, E), np.float32)
    for c in range(NCORES):
        o = res.results[c]["outd"]  # [T, E, R]
        out[c * BL : (c + 1) * BL] = o.reshape(T, E, BL, NB).transpose(2, 0, 3, 1)
    return out


kernel.last_exec_time_ns = None
kernel.last_trace = None


def _ensure_ntff_hook():
    """Register the axon NTFF profiling hook if the antenv shim module is
    missing in this image (the libaxon .so itself supports profiling)."""
    import sys
    import types

    try:
        from antenv.axon_hooks import get_axon_ntff_profile_hook  # noqa: F401

        return
    except ImportError:
        pass
    mod = types.ModuleType("antenv.axon_hooks")
    mod._hook = None

    def set_axon_ntff_profile_hook(h):
        mod._hook = h

    def get_axon_ntff_profile_hook():
        return mod._hook

    mod.set_axon_ntff_profile_hook = set_axon_ntff_profile_hook
    mod.get_axon_ntff_profile_hook = get_axon_ntff_profile_hook
    sys.modules["antenv.axon_hooks"] = mod
    try:
        from trn_agent_boot.trn_boot import _ntff_profile_via_ctypes

        hook = _ntff_profile_via_ctypes("/opt/axon/libaxon_pjrt.so")
        if hook is not None:
            mod._hook = hook
    except Exception:
        pass


# revision 29
# speedup vs baseline: 2.0370x; 1.2191x over previous
"""EntityNetwork recurrence kernel for 8 Trainium2 NeuronCores.

Sharding: data-parallel over batch (B=64 -> 8 per core); per core 160
independent entities r=(b,k) evolve a length-128 state over T=128 steps.

Design (v2, "r-layout"): the heavy per-entity scalars (gate g, inverse norm
iota) live on PARTITIONS so they are cheap per-partition DVE/ACT operands.

State per step t (all fp32):
  h_T  [E=128, R=160]  normalized current state, e-layout (PE stationary)
  u_A [128,128], u_B [32,128]  unnormalized state, r-layout (entities x E)
  iota [128, 2]        per-entity 1/||u|| (col 0: tile A, col 1: tile B)
with the invariant  h_cur = u * iota  and  h_T = (u * iota)^T.

Per timestep:
  pre_r = h @ U + (facts@W + keys@Vm + U_bias)    4 matmuls (lhsT = h_T / sel28)
  G     = h @ facts^T                             2 matmuls  [r, b]
  z     = sum_b (G + gbm) * onehot_b(r)           DVE STT + accum (gbm fold)
  g     = 1/(1+exp(-z))                           ACT Exp + DVE add/recip_fast
  hh*g  = Prelu(g * pre)                          ACT Prelu with scale=g col
  u'    = u * iota + hh*g                         DVE STT (normalize fold)
  n2    = sum_e u'^2                              STT + accum (gpsimd/DVE)
  iota' = exp(-0.5 * ln(n2 + eps))                ACT Ln + Exp
  h_T'  = u'^T @ diag(iota')                      2 transpose-matmuls
  out[t] = h_T'                                   DMA

Host-side precompute: keys_emb gather, facts@W fold, gw gate-bias fold,
mask fold (exact for binary masks), all O(T*B*E*E) ~ 0.27 GFLOP numpy.
ACT only ever uses Exp/Ln/Prelu/Copy -> a single table set, no reloads.
"""

import functools
import json
import os

import numpy as np


def _patch_act_tables():
    """Keep every ACT function this kernel uses (Sigmoid, Prelu, Square,
    Copy, Identity) only in the `sigmoid_and_others` table set, so bacc's
    table-load placement keeps ONE resident set and the kernel pays zero
    per-timestep ACT_TABLE_LOADs.  Set ids are untouched (entries keep
    their positions), so walrus/runtime still load the genuine set."""
    import functools as _ft

    import concourse.bacc as _bacc
    import concourse.hw_specs as _hw
    from concourse import mybir as _mb

    if getattr(_patch_act_tables, "_done", False):
        return
    AF = _mb.ActivationFunctionType
    mine = {AF.Exp, AF.Ln, AF.Prelu, AF.Square, AF.Copy, AF.Identity}
    orig = _hw.get_activation_tables

    @_ft.cache
    def patched(arch):
        out = {}
        for name, funcs in orig(arch).items():
            keepname = "natural_log_exp_and_others"
            out[name] = funcs if name == keepname else funcs - mine
        return out

    _hw.get_activation_tables = patched
    _bacc.get_activation_tables = patched
    _patch_act_tables._done = True


B, T, E, NB = 64, 128, 128, 20
NCORES = 8
BL = B // NCORES          # 8 stories per core
R = BL * NB               # 160 entities per core
RA = 128                  # tile A entities
RB = R - RA               # 32 tile B entities
S28 = BL + NB             # 28 = fW rows + C2 rows

# packa [128, PA]: U | stor_T | h0_T | h0_rA | sel8A | gbmA | I128 | eps
PA = E + T * BL + R + E + BL + T + E + 1
# packb [32, PB]: fwc2(28 rows) | sel28A(28) | sel28B(28) | h0_rB | sel8B | gbmB
PB = T * E + RA + RB + E + BL + T


@functools.lru_cache(maxsize=2)
def _program(alpha: float):
    from contextlib import ExitStack

    import concourse.bacc as bacc
    import concourse.tile as tile
    from concourse import mybir

    _patch_act_tables()

    f32 = mybir.dt.float32
    f32r = mybir.dt.float32r
    i32 = mybir.dt.int32
    AF = mybir.ActivationFunctionType
    ALU = mybir.AluOpType
    MAGIC = 0x5F3759DF

    nc = bacc.Bacc("TRN2", target_bir_lowering=False, debug=False)
    d_packa = nc.dram_tensor("packa", [E, PA], f32, kind="ExternalInput")
    d_packb = nc.dram_tensor("packb", [32, PB], f32, kind="ExternalInput")
    d_out = nc.dram_tensor("outd", [T, E, R], f32, kind="ExternalOutput")

    with ExitStack() as ctx:
        tc = ctx.enter_context(tile.TileContext(nc))
        consts = ctx.enter_context(tc.tile_pool(name="consts", bufs=1))
        hpool = ctx.enter_context(tc.tile_pool(name="hpool", bufs=3))
        upool = ctx.enter_context(tc.tile_pool(name="upool", bufs=2))
        work = ctx.enter_context(tc.tile_pool(name="work", bufs=2))
        psum = ctx.enter_context(tc.tile_pool(name="psum", bufs=1, space="PSUM"))

        sb_packa = consts.tile([E, PA], f32)
        nc.sync.dma_start(out=sb_packa, in_=d_packa[:, :])
        sb_packb = consts.tile([32, PB], f32)
        nc.sync.dma_start(out=sb_packb, in_=d_packb[:, :])

        o = 0
        sb_u_f = sb_packa[:, o : o + E]; o += E
        sb_stor_f = sb_packa[:, o : o + T * BL]; o += T * BL
        sb_h0T = sb_packa[:, o : o + R]; o += R
        sb_h0rA = sb_packa[:, o : o + E]; o += E
        sb_sel8A = sb_packa[:, o : o + BL]; o += BL
        sb_gbmA = sb_packa[:, o : o + T]; o += T
        sb_I_f = sb_packa[:, o : o + E]; o += E
        sb_eps = sb_packa[:, o : o + 1]; o += 1
        assert o == PA

        o = 0
        sb_fwc2_f = sb_packb[0:S28, o : o + T * E]; o += T * E
        sb_sel28A_f = sb_packb[0:S28, o : o + RA]; o += RA
        sb_sel28B_f = sb_packb[0:S28, o : o + RB]; o += RB
        sb_h0rB = sb_packb[:, o : o + E]; o += E
        sb_sel8B = sb_packb[:, o : o + BL]; o += BL
        sb_gbmB = sb_packb[:, o : o + T]; o += T
        assert o == PB

        # f32r (single-pass matmul) copies of the constant matmul operands
        sb_u = consts.tile([E, E], f32r, name="sb_u")
        nc.vector.tensor_copy(sb_u, sb_u_f)
        sb_stor = consts.tile([E, T * BL], f32r, name="sb_stor")
        nc.vector.tensor_copy(sb_stor, sb_stor_f)
        sb_I = consts.tile([E, E], f32r, name="sb_I")
        nc.vector.tensor_copy(sb_I, sb_I_f)
        sb_fwc2 = consts.tile([S28, T * E], f32r, name="sb_fwc2")
        nc.vector.tensor_copy(sb_fwc2, sb_fwc2_f)
        sb_sel28A = consts.tile([S28, RA], f32r, name="sb_sel28A")
        nc.vector.tensor_copy(sb_sel28A, sb_sel28A_f)
        sb_sel28B = consts.tile([S28, RB], f32r, name="sb_sel28B")
        nc.vector.tensor_copy(sb_sel28B, sb_sel28B_f)

        # initial state
        h_T = hpool.tile([E, R], f32r, name="h_T", tag="hT")
        nc.vector.tensor_copy(h_T, sb_h0T)
        u_A = upool.tile([RA, E], f32r, name="u_A", tag="uA")
        nc.vector.tensor_copy(u_A, sb_h0rA)
        u_B = upool.tile([RB, E], f32r, name="u_B", tag="uB")
        nc.vector.tensor_copy(u_B, sb_h0rB[0:RB, :])
        iota = upool.tile([RA, 2], f32, name="iota", tag="iota")
        nc.vector.memset(iota, 1.0)

        for t in range(T):
            ts_e = slice(t * E, (t + 1) * E)
            ts_b = slice(t * BL, (t + 1) * BL)

            # ---- pre-activation + gate dot products on PE
            preA = psum.tile([RA, E], f32, name="preA", tag="preA")
            preB = psum.tile([RB, E], f32, name="preB", tag="preB")
            GA = psum.tile([RA, BL], f32, name="GA", tag="GA")
            GB = psum.tile([RB, BL], f32, name="GB", tag="GB")
            nc.tensor.matmul(preA, h_T[:, 0:RA], sb_u, start=True, stop=False)
            nc.tensor.matmul(GA, h_T[:, 0:RA], sb_stor[:, ts_b], start=True, stop=True)
            nc.tensor.matmul(preB, h_T[:, RA:R], sb_u, start=True, stop=False)
            nc.tensor.matmul(GB, h_T[:, RA:R], sb_stor[:, ts_b], start=True, stop=True)
            nc.tensor.matmul(preA, sb_sel28A, sb_fwc2[:, ts_e], start=False, stop=True)
            nc.tensor.matmul(preB, sb_sel28B, sb_fwc2[:, ts_e], start=False, stop=True)

            # ---- gate: z = sum_b (G + gbm) * onehot ; g = sigmoid(z)
            zc = work.tile([RA, 2], f32, name="zc", tag="zc")
            junk8A = work.tile([RA, BL], f32, name="junk8A", tag="junk8A")
            junk8B = work.tile([RB, BL], f32, name="junk8B", tag="junk8B")
            nc.vector.scalar_tensor_tensor(
                out=junk8A, in0=GA, scalar=sb_gbmA[:, t : t + 1], in1=sb_sel8A,
                op0=ALU.add, op1=ALU.mult, accum_out=zc[:, 0:1],
            )
            nc.vector.scalar_tensor_tensor(
                out=junk8B, in0=GB, scalar=sb_gbmB[0:RB, t : t + 1],
                in1=sb_sel8B[0:RB, :],
                op0=ALU.add, op1=ALU.mult, accum_out=zc[0:RB, 1:2],
            )
            ez = work.tile([RA, 2], f32, name="ez", tag="ez")
            nc.scalar.activation(ez, zc, AF.Exp, scale=-1.0)
            g2 = work.tile([RA, 2], f32, name="g2", tag="g2")
            nc.vector.tensor_scalar_add(out=ez, in0=ez, scalar1=1.0)
            nc.vector.reciprocal_approx_fast(g2, ez)

            # ---- gated candidate: hh*g = Prelu(g * pre)
            hhgA = work.tile([RA, E], f32, name="hhgA", tag="hhgA")
            nc.scalar.activation(hhgA, preA, AF.Prelu, scale=g2[:, 0:1], alpha=alpha)
            hhgB = work.tile([RB, E], f32, name="hhgB", tag="hhgB")
            nc.scalar.activation(
                hhgB, preB, AF.Prelu, scale=g2[0:RB, 1:2], alpha=alpha
            )

            # ---- state update u' = u*iota + hh*g ; n2 = sum u'^2
            u_An = upool.tile([RA, E], f32r, name="u_A", tag="uA")
            nc.vector.scalar_tensor_tensor(
                out=u_An, in0=u_A, scalar=iota[:, 0:1], in1=hhgA,
                op0=ALU.mult, op1=ALU.add,
            )
            u_Bn = upool.tile([RB, E], f32r, name="u_B", tag="uB")
            nc.vector.scalar_tensor_tensor(
                out=u_Bn, in0=u_B, scalar=iota[0:RB, 1:2], in1=hhgB,
                op0=ALU.mult, op1=ALU.add,
            )
            n2c = work.tile([RA, 2], f32, name="n2c", tag="n2c")
            junkA = work.tile([RA, E], f32, name="junkA", tag="junkA")
            junkB = work.tile([RB, E], f32, name="junkB", tag="junkB")
            nc.vector.scalar_tensor_tensor(
                out=junkA, in0=u_An, scalar=1.0, in1=u_An,
                op0=ALU.mult, op1=ALU.mult, accum_out=n2c[:, 0:1],
            )
            nc.vector.scalar_tensor_tensor(
                out=junkB, in0=u_Bn, scalar=1.0, in1=u_Bn,
                op0=ALU.mult, op1=ALU.mult, accum_out=n2c[0:RB, 1:2],
            )

            # ---- iota' = rsqrt(n2 + eps) = exp(-0.5 ln(n2 + eps))
            ln2 = work.tile([RA, 2], f32, name="ln2", tag="ln2")
            nc.scalar.activation(ln2, n2c, AF.Ln, bias=sb_eps)
            iota_n = upool.tile([RA, 2], f32, name="iota", tag="iota")
            nc.scalar.activation(iota_n, ln2, AF.Exp, scale=-0.5)

            # ---- h_T' = u'^T @ diag(iota') (transpose + normalize fused)
            dmA = work.tile([RA, RA], f32r, name="dmA", tag="dmA")
            nc.vector.tensor_scalar_mul(out=dmA, in0=sb_I, scalar1=iota_n[:, 0:1])
            dmB = work.tile([RB, RB], f32r, name="dmB", tag="dmB")
            nc.vector.tensor_scalar_mul(
                out=dmB, in0=sb_I[0:RB, 0:RB], scalar1=iota_n[0:RB, 1:2]
            )
            trA = psum.tile([E, RA], f32, name="trA", tag="trA")
            nc.tensor.matmul(trA, u_An, dmA, start=True, stop=True)
            trB = psum.tile([E, RB], f32, name="trB", tag="trB")
            nc.tensor.matmul(trB, u_Bn, dmB, start=True, stop=True)

            h_Tn = hpool.tile([E, R], f32r, name="h_T", tag="hT")
            nc.scalar.copy(h_Tn[:, 0:RA], trA)
            nc.vector.tensor_copy(h_Tn[:, RA:R], trB)

            nc.sync.dma_start(out=d_out[t], in_=h_Tn.bitcast(f32))

            h_T, u_A, u_B, iota = h_Tn, u_An, u_Bn, iota_n

    nc.compile()
    return nc


def _host_prep(stories, mask, ke, g_bias, U, U_bias, Vm, W):
    """Build the per-core device input maps (packa/packb)."""
    C2 = ke @ Vm + U_bias[None, :]  # [NB, E]
    # selector matrices
    sel28 = np.zeros((S28, R), np.float32)
    sel8 = np.zeros((R, BL), np.float32)
    for b in range(BL):
        for k in range(NB):
            r = b * NB + k
            sel28[b, r] = 1.0
            sel28[BL + k, r] = 1.0
            sel8[r, b] = 1.0
    h0T = np.tile(ke.T, (1, BL)).astype(np.float32)        # [E, R]
    h0r = h0T.T.copy()                                     # [R, E]
    ident = np.eye(E, dtype=np.float32)
    u_dev = np.ascontiguousarray(U, np.float32)
    epscol = np.full((E, 1), 1e-24, np.float32)

    in_maps = []
    for c in range(NCORES):
        sl = slice(c * BL, (c + 1) * BL)
        st_c = stories[sl]  # [BL, T, E]
        m_c = mask[sl]      # [BL, T]
        fW = np.einsum("bte,ef->tbf", st_c, W)  # [T, BL, E]
        fwc2 = np.concatenate(
            [fW, np.broadcast_to(C2[None], (T, NB, E))], axis=1
        )  # [T, S28, E]
        fwc2_dev = np.ascontiguousarray(
            fwc2.transpose(1, 0, 2).reshape(S28, T * E), np.float32
        )
        gw = np.einsum("ke,bte->tbk", ke, st_c)  # [T, BL, NB]
        gbm = (
            g_bias[None, None, :] + gw + (m_c.T[:, :, None] - 1.0) * 1e9
        ).reshape(T, R).T  # [R, T]
        gbm = np.ascontiguousarray(gbm, np.float32)
        stor_dev = np.ascontiguousarray(
            st_c.transpose(2, 1, 0).reshape(E, T * BL), np.float32
        )
        packa = np.concatenate(
            [
                u_dev, stor_dev, h0T, h0r[0:RA], sel8[0:RA],
                gbm[0:RA], ident, epscol,
            ],
            axis=1,
        )
        pb = np.zeros((32, PB), np.float32)
        o = 0
        pb[0:S28, o : o + T * E] = fwc2_dev; o += T * E
        pb[0:S28, o : o + RA] = sel28[:, 0:RA]; o += RA
        pb[0:S28, o : o + RB] = sel28[:, RA:R]; o += RB
        pb[0:RB, o : o + E] = h0r[RA:R]; o += E
        pb[0:RB, o : o + BL] = sel8[RA:R]; o += BL
        pb[0:RB, o : o + T] = gbm[RA:R]; o += T
        assert o == PB
        in_maps.append(
            {
                "packa": np.ascontiguousarray(packa, np.float32),
                "packb": pb,
            }
        )
    return in_maps


def kernel(
    stories,
    stories_mask,
    keys,
    embeddings,
    g_bias,
    U,
    U_bias,
    Vm,
    W,
    prelu_a,
):
    stories = np.asarray(stories, np.float32)
    mask = np.asarray(stories_mask, np.float32)
    keys = np.asarray(keys).astype(np.int64)
    emb = np.asarray(embeddings, np.float32)
    g_bias = np.asarray(g_bias, np.float32)
    U = np.asarray(U, np.float32)
    U_bias = np.asarray(U_bias, np.float32)
    Vm = np.asarray(Vm, np.float32)
    W = np.asarray(W, np.float32)
    alpha = float(np.asarray(prelu_a))

    ke = emb[keys]  # [NB, E]
    in_maps = _host_prep(stories, mask, ke, g_bias, U, U_bias, Vm, W)

    nc = _program(alpha)
    from concourse.bass_utils import run_bass_kernel_spmd

    trace = bool(int(os.environ.get("KBENCH_TRACE", "0")))
    if trace:
        _ensure_ntff_hook()
    res = run_bass_kernel_spmd(
        nc, in_maps, core_ids=list(range(NCORES)), trace=trace
    )
    if trace and res.exec_time_ns is not None:
        kernel.last_exec_time_ns = res.exec_time_ns
        kernel.last_trace = res.instructions_and_trace
    out = np.empty((B, T, NB, E), np.float32)
    for c in range(NCORES):
        o = res.results[c]["outd"]  # [T, E, R]
        out[c * BL : (c + 1) * BL] = o.reshape(T, E, BL, NB).transpose(2, 0, 3, 1)
    return out


kernel.last_exec_time_ns = None
kernel.last_trace = None


def _ensure_ntff_hook():
    """Register the axon NTFF profiling hook if the antenv shim module is
    missing in this image (the libaxon .so itself supports profiling)."""
    import sys
    import types

    try:
        from antenv.axon_hooks import get_axon_ntff_profile_hook  # noqa: F401

        return
    except ImportError:
        pass
    mod = types.ModuleType("antenv.axon_hooks")
    mod._hook = None

    def set_axon_ntff_profile_hook(h):
        mod._hook = h

    def get_axon_ntff_profile_hook():
        return mod._hook

    mod.set_axon_ntff_profile_hook = set_axon_ntff_profile_hook
    mod.get_axon_ntff_profile_hook = get_axon_ntff_profile_hook
    sys.modules["antenv.axon_hooks"] = mod
    try:
        from trn_agent_boot.trn_boot import _ntff_profile_via_ctypes

        hook = _ntff_profile_via_ctypes("/opt/axon/libaxon_pjrt.so")
        if hook is not None:
            mod._hook = hook
    except Exception:
        pass


# revision 30
# speedup vs baseline: 2.1586x; 1.0597x over previous
"""EntityNetwork recurrence kernel for 8 Trainium2 NeuronCores.

Sharding: data-parallel over batch (B=64 -> 8 per core); per core 160
independent entities r=(b,k) evolve a length-128 state over T=128 steps.

Design (v2, "r-layout"): the heavy per-entity scalars (gate g, inverse norm
iota) live on PARTITIONS so they are cheap per-partition DVE/ACT operands.

State per step t (all fp32):
  h_T  [E=128, R=160]  normalized current state, e-layout (PE stationary)
  u_A [128,128], u_B [32,128]  unnormalized state, r-layout (entities x E)
  iota [128, 2]        per-entity 1/||u|| (col 0: tile A, col 1: tile B)
with the invariant  h_cur = u * iota  and  h_T = (u * iota)^T.

Per timestep:
  pre_r = h @ U + (facts@W + keys@Vm + U_bias)    4 matmuls (lhsT = h_T / sel28)
  G     = h @ facts^T                             2 matmuls  [r, b]
  z     = sum_b (G + gbm) * onehot_b(r)           DVE STT + accum (gbm fold)
  g     = 1/(1+exp(-z))                           ACT Exp + DVE add/recip_fast
  hh*g  = Prelu(g * pre)                          ACT Prelu with scale=g col
  u'    = u * iota + hh*g                         DVE STT (normalize fold)
  n2    = sum_e u'^2                              STT + accum (gpsimd/DVE)
  iota' = exp(-0.5 * ln(n2 + eps))                ACT Ln + Exp
  h_T'  = u'^T @ diag(iota')                      2 transpose-matmuls
  out[t] = h_T'                                   DMA

Host-side precompute: keys_emb gather, facts@W fold, gw gate-bias fold,
mask fold (exact for binary masks), all O(T*B*E*E) ~ 0.27 GFLOP numpy.
ACT only ever uses Exp/Ln/Prelu/Copy -> a single table set, no reloads.
"""

import functools
import json
import os

import numpy as np


def _patch_act_tables():
    """Keep every ACT function this kernel uses (Sigmoid, Prelu, Square,
    Copy, Identity) only in the `sigmoid_and_others` table set, so bacc's
    table-load placement keeps ONE resident set and the kernel pays zero
    per-timestep ACT_TABLE_LOADs.  Set ids are untouched (entries keep
    their positions), so walrus/runtime still load the genuine set."""
    import functools as _ft

    import concourse.bacc as _bacc
    import concourse.hw_specs as _hw
    from concourse import mybir as _mb

    if getattr(_patch_act_tables, "_done", False):
        return
    AF = _mb.ActivationFunctionType
    mine = {AF.Exp, AF.Ln, AF.Prelu, AF.Square, AF.Copy, AF.Identity}
    orig = _hw.get_activation_tables

    @_ft.cache
    def patched(arch):
        out = {}
        for name, funcs in orig(arch).items():
            keepname = "natural_log_exp_and_others"
            out[name] = funcs if name == keepname else funcs - mine
        return out

    _hw.get_activation_tables = patched
    _bacc.get_activation_tables = patched
    _patch_act_tables._done = True


B, T, E, NB = 64, 128, 128, 20
NCORES = 8
BL = B // NCORES          # 8 stories per core
R = BL * NB               # 160 entities per core
RA = 128                  # tile A entities
RB = R - RA               # 32 tile B entities
S28 = BL + NB             # 28 = fW rows + C2 rows

# packa [128, PA]: U | stor_T | h0_T | h0_rA | sel8A | gbmA | I128 | eps
PA = E + T * BL + R + E + BL + T + E + 1
# packb [32, PB]: fwc2(28 rows) | sel28A(28) | sel28B(28) | h0_rB | sel8B | gbmB
PB = T * E + RA + RB + E + BL + T


@functools.lru_cache(maxsize=2)
def _program(alpha: float):
    from contextlib import ExitStack

    import concourse.bacc as bacc
    import concourse.tile as tile
    from concourse import mybir

    _patch_act_tables()

    f32 = mybir.dt.float32
    f32r = mybir.dt.float32r
    i32 = mybir.dt.int32
    AF = mybir.ActivationFunctionType
    ALU = mybir.AluOpType
    MAGIC = 0x5F3759DF

    nc = bacc.Bacc("TRN2", target_bir_lowering=False, debug=False)
    d_packa = nc.dram_tensor("packa", [E, PA], f32, kind="ExternalInput")
    d_packb = nc.dram_tensor("packb", [32, PB], f32, kind="ExternalInput")
    d_out = nc.dram_tensor("outd", [T, E, R], f32, kind="ExternalOutput")

    with ExitStack() as ctx:
        tc = ctx.enter_context(tile.TileContext(nc))
        consts = ctx.enter_context(tc.tile_pool(name="consts", bufs=1))
        hpool = ctx.enter_context(tc.tile_pool(name="hpool", bufs=3))
        upool = ctx.enter_context(tc.tile_pool(name="upool", bufs=3))
        work = ctx.enter_context(tc.tile_pool(name="work", bufs=3))
        psum = ctx.enter_context(tc.tile_pool(name="psum", bufs=1, space="PSUM"))

        sb_packa = consts.tile([E, PA], f32)
        nc.sync.dma_start(out=sb_packa, in_=d_packa[:, :])
        sb_packb = consts.tile([32, PB], f32)
        nc.sync.dma_start(out=sb_packb, in_=d_packb[:, :])

        o = 0
        sb_u_f = sb_packa[:, o : o + E]; o += E
        sb_stor_f = sb_packa[:, o : o + T * BL]; o += T * BL
        sb_h0T = sb_packa[:, o : o + R]; o += R
        sb_h0rA = sb_packa[:, o : o + E]; o += E
        sb_sel8A = sb_packa[:, o : o + BL]; o += BL
        sb_gbmA = sb_packa[:, o : o + T]; o += T
        sb_I_f = sb_packa[:, o : o + E]; o += E
        sb_eps = sb_packa[:, o : o + 1]; o += 1
        assert o == PA

        o = 0
        sb_fwc2_f = sb_packb[0:S28, o : o + T * E]; o += T * E
        sb_sel28A_f = sb_packb[0:S28, o : o + RA]; o += RA
        sb_sel28B_f = sb_packb[0:S28, o : o + RB]; o += RB
        sb_h0rB = sb_packb[:, o : o + E]; o += E
        sb_sel8B = sb_packb[:, o : o + BL]; o += BL
        sb_gbmB = sb_packb[:, o : o + T]; o += T
        assert o == PB

        # f32r (single-pass matmul) copies of the constant matmul operands
        sb_u = consts.tile([E, E], f32r, name="sb_u")
        nc.vector.tensor_copy(sb_u, sb_u_f)
        sb_stor = consts.tile([E, T * BL], f32r, name="sb_stor")
        nc.vector.tensor_copy(sb_stor, sb_stor_f)
        sb_I = consts.tile([E, E], f32r, name="sb_I")
        nc.vector.tensor_copy(sb_I, sb_I_f)
        sb_fwc2 = consts.tile([S28, T * E], f32r, name="sb_fwc2")
        nc.vector.tensor_copy(sb_fwc2, sb_fwc2_f)
        sb_sel28A = consts.tile([S28, RA], f32r, name="sb_sel28A")
        nc.vector.tensor_copy(sb_sel28A, sb_sel28A_f)
        sb_sel28B = consts.tile([S28, RB], f32r, name="sb_sel28B")
        nc.vector.tensor_copy(sb_sel28B, sb_sel28B_f)

        # initial state
        h_T = hpool.tile([E, R], f32r, name="h_T", tag="hT")
        nc.vector.tensor_copy(h_T, sb_h0T)
        u_A = upool.tile([RA, E], f32r, name="u_A", tag="uA")
        nc.vector.tensor_copy(u_A, sb_h0rA)
        u_B = upool.tile([RB, E], f32r, name="u_B", tag="uB")
        nc.vector.tensor_copy(u_B, sb_h0rB[0:RB, :])
        iotaA = upool.tile([RA, 1], f32, name="iotaA", tag="iotaA")
        nc.vector.memset(iotaA, 1.0)
        iotaB = upool.tile([RB, 1], f32, name="iotaB", tag="iotaB")
        nc.vector.memset(iotaB, 1.0)

        for t in range(T):
            ts_e = slice(t * E, (t + 1) * E)
            ts_b = slice(t * BL, (t + 1) * BL)

            # ---- pre-activation + gate dot products on PE
            preA = psum.tile([RA, E], f32, name="preA", tag="preA")
            preB = psum.tile([RB, E], f32, name="preB", tag="preB")
            GA = psum.tile([RA, BL], f32, name="GA", tag="GA")
            GB = psum.tile([RB, BL], f32, name="GB", tag="GB")
            nc.tensor.matmul(preA, h_T[:, 0:RA], sb_u, start=True, stop=False)
            nc.tensor.matmul(GA, h_T[:, 0:RA], sb_stor[:, ts_b], start=True, stop=True)
            nc.tensor.matmul(preB, h_T[:, RA:R], sb_u, start=True, stop=False)
            nc.tensor.matmul(GB, h_T[:, RA:R], sb_stor[:, ts_b], start=True, stop=True)
            nc.tensor.matmul(preA, sb_sel28A, sb_fwc2[:, ts_e], start=False, stop=True)
            nc.tensor.matmul(preB, sb_sel28B, sb_fwc2[:, ts_e], start=False, stop=True)

            # ---- gate: z = sum_b (G + gbm) * onehot ; g = sigmoid(z)
            zcA = work.tile([RA, 1], f32, name="zcA", tag="zcA")
            zcB = work.tile([RB, 1], f32, name="zcB", tag="zcB")
            junk8A = work.tile([RA, BL], f32, name="junk8A", tag="junk8A")
            junk8B = work.tile([RB, BL], f32, name="junk8B", tag="junk8B")
            nc.vector.scalar_tensor_tensor(
                out=junk8A, in0=GA, scalar=sb_gbmA[:, t : t + 1], in1=sb_sel8A,
                op0=ALU.add, op1=ALU.mult, accum_out=zcA,
            )
            nc.vector.scalar_tensor_tensor(
                out=junk8B, in0=GB, scalar=sb_gbmB[0:RB, t : t + 1],
                in1=sb_sel8B[0:RB, :],
                op0=ALU.add, op1=ALU.mult, accum_out=zcB,
            )
            ezA = work.tile([RA, 1], f32, name="ezA", tag="ezA")
            nc.scalar.activation(ezA, zcA, AF.Exp, scale=-1.0)
            gA = work.tile([RA, 1], f32, name="gA", tag="gA")
            nc.vector.tensor_scalar_add(out=ezA, in0=ezA, scalar1=1.0)
            nc.vector.reciprocal_approx_fast(gA, ezA)
            ezB = work.tile([RB, 1], f32, name="ezB", tag="ezB")
            nc.scalar.activation(ezB, zcB, AF.Exp, scale=-1.0)
            gB = work.tile([RB, 1], f32, name="gB", tag="gB")
            nc.vector.tensor_scalar_add(out=ezB, in0=ezB, scalar1=1.0)
            nc.vector.reciprocal_approx_fast(gB, ezB)

            # ---- gated candidate: hh*g = Prelu(g * pre)
            hhgA = work.tile([RA, E], f32, name="hhgA", tag="hhgA")
            nc.scalar.activation(hhgA, preA, AF.Prelu, scale=gA, alpha=alpha)
            hhgB = work.tile([RB, E], f32, name="hhgB", tag="hhgB")
            nc.scalar.activation(hhgB, preB, AF.Prelu, scale=gB, alpha=alpha)

            # ---- state update u' = u*iota + hh*g ; n2 = sum u'^2
            u_An = upool.tile([RA, E], f32r, name="u_A", tag="uA")
            nc.vector.scalar_tensor_tensor(
                out=u_An, in0=u_A, scalar=iotaA, in1=hhgA,
                op0=ALU.mult, op1=ALU.add,
            )
            u_Bn = upool.tile([RB, E], f32r, name="u_B", tag="uB")
            nc.vector.scalar_tensor_tensor(
                out=u_Bn, in0=u_B, scalar=iotaB, in1=hhgB,
                op0=ALU.mult, op1=ALU.add,
            )
            n2cA = work.tile([RA, 1], f32, name="n2cA", tag="n2cA")
            n2cB = work.tile([RB, 1], f32, name="n2cB", tag="n2cB")
            junkA = work.tile([RA, E], f32, name="junkA", tag="junkA")
            junkB = work.tile([RB, E], f32, name="junkB", tag="junkB")
            nc.vector.scalar_tensor_tensor(
                out=junkA, in0=u_An, scalar=1.0, in1=u_An,
                op0=ALU.mult, op1=ALU.mult, accum_out=n2cA,
            )
            nc.vector.scalar_tensor_tensor(
                out=junkB, in0=u_Bn, scalar=1.0, in1=u_Bn,
                op0=ALU.mult, op1=ALU.mult, accum_out=n2cB,
            )

            # ---- iota' = rsqrt(n2 + eps) = exp(-0.5 ln(n2 + eps))
            lnA = work.tile([RA, 1], f32, name="lnA", tag="lnA")
            nc.scalar.activation(lnA, n2cA, AF.Ln, bias=sb_eps)
            iotaA_n = upool.tile([RA, 1], f32, name="iotaA", tag="iotaA")
            nc.scalar.activation(iotaA_n, lnA, AF.Exp, scale=-0.5)
            lnB = work.tile([RB, 1], f32, name="lnB", tag="lnB")
            nc.scalar.activation(lnB, n2cB, AF.Ln, bias=sb_eps[0:RB, :])
            iotaB_n = upool.tile([RB, 1], f32, name="iotaB", tag="iotaB")
            nc.scalar.activation(iotaB_n, lnB, AF.Exp, scale=-0.5)

            # ---- h_T' = u'^T @ diag(iota') (transpose + normalize fused)
            dmA = work.tile([RA, RA], f32r, name="dmA", tag="dmA")
            nc.vector.tensor_scalar_mul(out=dmA, in0=sb_I, scalar1=iotaA_n)
            dmB = work.tile([RB, RB], f32r, name="dmB", tag="dmB")
            nc.vector.tensor_scalar_mul(
                out=dmB, in0=sb_I[0:RB, 0:RB], scalar1=iotaB_n
            )
            trA = psum.tile([E, RA], f32, name="trA", tag="trA")
            nc.tensor.matmul(trA, u_An, dmA, start=True, stop=True)
            trB = psum.tile([E, RB], f32, name="trB", tag="trB")
            nc.tensor.matmul(trB, u_Bn, dmB, start=True, stop=True)

            h_Tn = hpool.tile([E, R], f32r, name="h_T", tag="hT")
            nc.scalar.copy(h_Tn[:, 0:RA], trA)
            nc.vector.tensor_copy(h_Tn[:, RA:R], trB)

            nc.sync.dma_start(out=d_out[t], in_=h_Tn.bitcast(f32))

            h_T, u_A, u_B = h_Tn, u_An, u_Bn
            iotaA, iotaB = iotaA_n, iotaB_n

    nc.compile()
    return nc


def _host_prep(stories, mask, ke, g_bias, U, U_bias, Vm, W):
    """Build the per-core device input maps (packa/packb)."""
    C2 = ke @ Vm + U_bias[None, :]  # [NB, E]
    # selector matrices
    sel28 = np.zeros((S28, R), np.float32)
    sel8 = np.zeros((R, BL), np.float32)
    for b in range(BL):
        for k in range(NB):
            r = b * NB + k
            sel28[b, r] = 1.0
            sel28[BL + k, r] = 1.0
            sel8[r, b] = 1.0
    h0T = np.tile(ke.T, (1, BL)).astype(np.float32)        # [E, R]
    h0r = h0T.T.copy()                                     # [R, E]
    ident = np.eye(E, dtype=np.float32)
    u_dev = np.ascontiguousarray(U, np.float32)
    epscol = np.full((E, 1), 1e-24, np.float32)

    in_maps = []
    for c in range(NCORES):
        sl = slice(c * BL, (c + 1) * BL)
        st_c = stories[sl]  # [BL, T, E]
        m_c = mask[sl]      # [BL, T]
        fW = np.einsum("bte,ef->tbf", st_c, W)  # [T, BL, E]
        fwc2 = np.concatenate(
            [fW, np.broadcast_to(C2[None], (T, NB, E))], axis=1
        )  # [T, S28, E]
        fwc2_dev = np.ascontiguousarray(
            fwc2.transpose(1, 0, 2).reshape(S28, T * E), np.float32
        )
        gw = np.einsum("ke,bte->tbk", ke, st_c)  # [T, BL, NB]
        gbm = (
            g_bias[None, None, :] + gw + (m_c.T[:, :, None] - 1.0) * 1e9
        ).reshape(T, R).T  # [R, T]
        gbm = np.ascontiguousarray(gbm, np.float32)
        stor_dev = np.ascontiguousarray(
            st_c.transpose(2, 1, 0).reshape(E, T * BL), np.float32
        )
        packa = np.concatenate(
            [
                u_dev, stor_dev, h0T, h0r[0:RA], sel8[0:RA],
                gbm[0:RA], ident, epscol,
            ],
            axis=1,
        )
        pb = np.zeros((32, PB), np.float32)
        o = 0
        pb[0:S28, o : o + T * E] = fwc2_dev; o += T * E
        pb[0:S28, o : o + RA] = sel28[:, 0:RA]; o += RA
        pb[0:S28, o : o + RB] = sel28[:, RA:R]; o += RB
        pb[0:RB, o : o + E] = h0r[RA:R]; o += E
        pb[0:RB, o : o + BL] = sel8[RA:R]; o += BL
        pb[0:RB, o : o + T] = gbm[RA:R]; o += T
        assert o == PB
        in_maps.append(
            {
                "packa": np.ascontiguousarray(packa, np.float32),
                "packb": pb,
            }
        )
    return in_maps


def kernel(
    stories,
    stories_mask,
    keys,
    embeddings,
    g_bias,
    U,
    U_bias,
    Vm,
    W,
    prelu_a,
):
    stories = np.asarray(stories, np.float32)
    mask = np.asarray(stories_mask, np.float32)
    keys = np.asarray(keys).astype(np.int64)
    emb = np.asarray(embeddings, np.float32)
    g_bias = np.asarray(g_bias, np.float32)
    U = np.asarray(U, np.float32)
    U_bias = np.asarray(U_bias, np.float32)
    Vm = np.asarray(Vm, np.float32)
    W = np.asarray(W, np.float32)
    alpha = float(np.asarray(prelu_a))

    ke = emb[keys]  # [NB, E]
    in_maps = _host_prep(stories, mask, ke, g_bias, U, U_bias, Vm, W)

    nc = _program(alpha)
    from concourse.bass_utils import run_bass_kernel_spmd

    trace = bool(int(os.environ.get("KBENCH_TRACE", "0")))
    if trace:
        _ensure_ntff_hook()
    res = run_bass_kernel_spmd(
        nc, in_maps, core_ids=list(range(NCORES)), trace=trace
    )
    if trace and res.exec_time_ns is not None:
        kernel.last_exec_time_ns = res.exec_time_ns
        kernel.last_trace = res.instructions_and_trace
    out = np.empty((B, T, NB, E), np.float32)
    for c in range(NCORES):
        o = res.results[c]["outd"]  # [T, E, R]
        out[c * BL : (c + 1) * BL] = o.reshape(T, E, BL, NB).transpose(2, 0, 3, 1)
    return out


kernel.last_exec_time_ns = None
kernel.last_trace = None


def _ensure_ntff_hook():
    """Register the axon NTFF profiling hook if the antenv shim module is
    missing in this image (the libaxon .so itself supports profiling)."""
    import sys
    import types

    try:
        from antenv.axon_hooks import get_axon_ntff_profile_hook  # noqa: F401

        return
    except ImportError:
        pass
    mod = types.ModuleType("antenv.axon_hooks")
    mod._hook = None

    def set_axon_ntff_profile_hook(h):
        mod._hook = h

    def get_axon_ntff_profile_hook():
        return mod._hook

    mod.set_axon_ntff_profile_hook = set_axon_ntff_profile_hook
    mod.get_axon_ntff_profile_hook = get_axon_ntff_profile_hook
    sys.modules["antenv.axon_hooks"] = mod
    try:
        from trn_agent_boot.trn_boot import _ntff_profile_via_ctypes

        hook = _ntff_profile_via_ctypes("/opt/axon/libaxon_pjrt.so")
        if hook is not None:
            mod._hook = hook
    except Exception:
        pass
